# revision 1
# baseline (speedup 1.0000x reference)
"""Bidirectional 2-layer ConvLSTM (3x3 grid) + FC head, Trainium2 Bass kernel.

Sharding: data-parallel over batch. B=64 across 8 cores -> 8 batches/core.
Weights replicated; no inter-core communication.

Per-core pipeline (single NEFF):
  A) transpose x to channel-major (PE transpose), layer-0 input projections
     for both directions as bf16 tap-accumulated matmuls -> DRAM zx0
  B) layer-0 recurrence, fwd+bwd chains interleaved per step; conv(h) via
     9-tap matmuls from a zero-padded h tile; gates on ACT/DVE
  C) layer-1 input projections from h0 = hf0 + hb0 -> DRAM zx1
  D) layer-1 recurrence (same as B)
  E) FC head on TensorE -> out [7, T*BL]
"""

import numpy as np
import ml_dtypes

import concourse.bass as bass
import concourse.mybir as mybir
from concourse.tile import TileContext
from concourse.masks import make_identity

BF16 = mybir.dt.bfloat16
F32 = mybir.dt.float32

B_FULL, T_FULL, C_IN, H, NCLS = 64, 128, 256, 128, 7
NCORES = 8
BL = B_FULL // NCORES  # local batch = 8
CLIPPED = True  # clipped-tap matmuls (multi-dim PSUM out APs, HW-validated)

# taps ordered center-first so the first matmul of each accumulation group
# covers every output column (has_written semantics)
TAPS = [(1, 1)] + [(dy, dx) for dy in range(3) for dx in range(3) if (dy, dx) != (1, 1)]


def _clip(d):
    # output-pixel range [p0, p0+n) and source range [s0, s0+n) for tap offset d
    if d == 0:
        return 1, 0, 2
    if d == 1:
        return 0, 0, 3
    return 0, 1, 2


def _patch_tile_drain():
    """This walrus rejects >1 sync wait on a Drain: keep the first wait on the
    drain and move the rest onto single-wait NOPs executed just before it."""
    from bass_rust import ScopedClock

    if getattr(TileContext, "_drain_patched", False):
        return

    def _drain_and_barrier(self, tick_clock, wait_clock):
        nc = self.nc
        drain_inst = nc.sync.drain()
        wait_clock.add_sem_waits(
            drain_inst.ins, ScopedClock({None: tick_clock.global_clock})
        )
        si = drain_inst.ins.sync_info
        waits = list(si.on_wait)
        if len(waits) > 1:
            while len(si.on_wait) > 1:
                si.on_wait.pop()
            for w in waits[1:]:
                nop = nc.sync.nop()
                nop.ins.sync_info = mybir.SyncInfo(on_wait=[w], on_update=[])
        nc.all_engine_barrier()
        assert self.sems is not None
        popped = nc._tile_sem_poison_stack.pop()
        assert popped is self._sem_poison
        nc.clear_and_free_semaphores(list(self.sems.allocated().values()))
        nc.all_engine_barrier()

    TileContext._drain_and_barrier = _drain_and_barrier
    TileContext._drain_patched = True


def _fix_multi_waits(raw):
    """This walrus accepts at most 1 sync wait per instruction (2 for
    EventSemaphore). Hoist excess waits onto single-wait EventSemaphore
    carriers inserted just before the instruction on the same engine."""
    import json

    d = json.loads(raw)
    nid = 0
    for fn in d["functions"]:
        for blk in fn["blocks"]:
            out = []
            for inst in blk["instructions"]:
                si = inst.get("sync_info")
                ow = (si or {}).get("on_wait") or []
                cap = 2 if inst.get("opcode") == "EventSemaphore" else 1
                if len(ow) > cap:
                    for w in ow[cap:]:
                        nid += 1
                        out.append({
                            "debug": inst.get("debug", 0),
                            "engine": inst["engine"],
                            "ins": [],
                            "name": f"I-xwait-{nid}",
                            "opcode": "EventSemaphore",
                            "outs": [],
                            "sync_info": {"on_update": [], "on_wait": [w]},
                        })
                    si["on_wait"] = ow[:cap]
                out.append(inst)
            blk["instructions"] = out
    return json.dumps(d).encode()


def build_program(T=T_FULL, phases="ABCDE"):
    """Build the per-core Bass program. Returns nc."""
    _patch_tile_drain()
    G = T * BL  # column groups, g = t*BL + b
    GT = 128 if G % 128 == 0 else G  # groups per transpose tile
    assert G % GT == 0 and GT % 32 == 0
    n_gt = G // GT
    n_sub = GT // 32

    nc = bass.Bass()

    # ---- I/O ----
    x = nc.dram_tensor("x", [BL, T, C_IN, 3, 3], F32, kind="ExternalInput")
    wx0 = {}
    wh0 = {}
    wx1 = {}
    wh1 = {}
    bias_in = {}
    for d in ("f", "b"):
        wx0[d] = nc.dram_tensor(f"wx0{d}", [128, 2, 9, 512], BF16, kind="ExternalInput")
        wh0[d] = nc.dram_tensor(f"wh0{d}", [128, 9, 512], BF16, kind="ExternalInput")
        wx1[d] = nc.dram_tensor(f"wx1{d}", [128, 9, 512], BF16, kind="ExternalInput")
        wh1[d] = nc.dram_tensor(f"wh1{d}", [128, 9, 512], BF16, kind="ExternalInput")
        bias_in[f"0{d}"] = nc.dram_tensor(f"bias0{d}", [128, 4], F32, kind="ExternalInput")
        bias_in[f"1{d}"] = nc.dram_tensor(f"bias1{d}", [128, 4], F32, kind="ExternalInput")
    fcw = nc.dram_tensor("fcw", [128, 9, NCLS], BF16, kind="ExternalInput")
    fcb = nc.dram_tensor("fcb", [NCLS, 1], F32, kind="ExternalInput")
    out = nc.dram_tensor("out", [NCLS, G], F32, kind="ExternalOutput")

    # ---- DRAM scratch ----
    zx0 = {d: nc.dram_tensor(f"zx0{d}", [4, 128, G, 9], F32) for d in ("f", "b")}
    zx1 = {d: nc.dram_tensor(f"zx1{d}", [4, 128, G, 9], F32) for d in ("f", "b")}
    h0d = {d: nc.dram_tensor(f"h0{d}", [128, G, 9], F32) for d in ("f", "b")}
    h1d = {d: nc.dram_tensor(f"h1{d}", [128, G, 9], F32) for d in ("f", "b")}

    with TileContext(nc) as tc:
        with tc.tile_pool(name="persist", bufs=1) as pp:
            # persistent weights in SBUF
            wx0_sb = {d: pp.tile([128, 2, 9, 512], BF16, name=f"wx0{d}", tag=f"wx0{d}") for d in ("f", "b")}
            wh0_sb = {d: pp.tile([128, 9, 512], BF16, name=f"wh0{d}", tag=f"wh0{d}") for d in ("f", "b")}
            wx1_sb = {d: pp.tile([128, 9, 512], BF16, name=f"wx1{d}", tag=f"wx1{d}") for d in ("f", "b")}
            wh1_sb = {d: pp.tile([128, 9, 512], BF16, name=f"wh1{d}", tag=f"wh1{d}") for d in ("f", "b")}
            bias_sb = {}
            for d in ("f", "b"):
                nc.sync.dma_start(out=wx0_sb[d][:], in_=wx0[d][:])
                nc.sync.dma_start(out=wh0_sb[d][:], in_=wh0[d][:])
                nc.sync.dma_start(out=wx1_sb[d][:], in_=wx1[d][:])
                nc.sync.dma_start(out=wh1_sb[d][:], in_=wh1[d][:])
                for l in ("0", "1"):
                    bias_sb[l + d] = pp.tile([128, 4], F32, name=f"bias{l}{d}", tag=f"bias{l}{d}")
                    nc.sync.dma_start(out=bias_sb[l + d][:], in_=bias_in[l + d][:])
            fcw_sb = pp.tile([128, 9, NCLS], BF16, tag="fcw")
            nc.sync.dma_start(out=fcw_sb[:], in_=fcw[:])
            fcb_sb = pp.tile([NCLS, 1], F32, tag="fcb")
            nc.sync.dma_start(out=fcb_sb[:], in_=fcb[:])
            ident = pp.tile([128, 128], F32, tag="ident")
            make_identity(nc, ident[:])

            # ================= Phase A: layer-0 x projections =================
            # xpad: channel-major zero-padded x, double-buffered over g-tiles.
            # Borders memset once; interiors fully rewritten each g-tile.
            xpad = [
                pp.tile([128, 2, GT, 5, 5], BF16, name=f"xpad{par}", tag=f"xpad{par}")
                for par in range(2)
            ]
            nc.gpsimd.memset(xpad[0][:], 0.0)
            nc.gpsimd.memset(xpad[1][:], 0.0)
            x_gv = x[:].rearrange("b t c y x -> t b (c y x)")  # (T, BL, 2304)
            with (
                tc.tile_pool(name="a_xg", bufs=2) as xgp,
                tc.tile_pool(name="a_tp", bufs=2, space="PSUM") as tpp,
                tc.tile_pool(name="a_zp", bufs=5, space="PSUM") as zpp,
                tc.tile_pool(name="a_zs", bufs=3) as zsp,
            ):
                for gt in range(n_gt if "A" in phases else 0):
                    t0 = gt * (GT // BL)
                    nt = GT // BL
                    xg = xgp.tile([GT, C_IN * 9], F32, tag="xg")
                    # one DMA per time-step: SBUF partition dim must stay a
                    # single dim (split-partition DMA misbehaves on HW)
                    for ts in range(nt):
                        nc.sync.dma_start(
                            out=xg[ts * BL : (ts + 1) * BL, :],
                            in_=x_gv[t0 + ts],
                        )
                    xgv = xg[:].rearrange("g (c y x) -> g c y x", y=3, x=3)
                    xp = xpad[gt % 2]
                    for cb in range(2):
                        for y in range(3):
                            for xx in range(3):
                                tp = tpp.tile([128, GT], F32, tag="tp")
                                nc.tensor.transpose(
                                    tp[:], xgv[:, cb * 128 : (cb + 1) * 128, y, xx],
                                    ident[:GT, :GT],
                                )
                                nc.vector.tensor_copy(
                                    xp[:, cb, :, 1 + y, 1 + xx], tp[:]
                                )
                    for sub in range(n_sub):
                        g0 = sub * 32
                        for d in ("f", "b"):
                            zs = zsp.tile([128, 4, 32, 9], F32, tag="zs")
                            for cb_o in range(4):
                                zp = zpp.tile([128, 32, 3, 3], F32, tag="zp")
                                zpf = zp[:].rearrange("p g y x -> p (g y x)")
                                k = 0
                                for dy, dx in TAPS:
                                    py, sy, ny = _clip(dy)
                                    px, sx, nx2 = _clip(dx)
                                    for cb_i in range(2):
                                        w_ap = wx0_sb[d][
                                            :, cb_i, dy * 3 + dx,
                                            cb_o * 128 : (cb_o + 1) * 128,
                                        ]
                                        if CLIPPED and not (ny == 3 and nx2 == 3):
                                            o_ap = zp[:, :, py : py + ny, px : px + nx2]
                                            r_ap = xp[
                                                :, cb_i, g0 : g0 + 32,
                                                1 + sy : 1 + sy + ny,
                                                1 + sx : 1 + sx + nx2,
                                            ]
                                        else:
                                            o_ap = zpf
                                            r_ap = xp[
                                                :, cb_i, g0 : g0 + 32,
                                                dy : dy + 3, dx : dx + 3,
                                            ]
                                        nc.tensor.matmul(
                                            o_ap, w_ap, r_ap,
                                            start=(k == 0),
                                            stop=(k == 17),
                                        )
                                        k += 1
                                nc.vector.tensor_copy(
                                    zs[:, cb_o],
                                    zp[:].rearrange("p g y x -> p g (y x)"),
                                )
                            nc.sync.dma_start(
                                out=zx0[d][
                                    :, :, gt * GT + g0 : gt * GT + g0 + 32, :
                                ].rearrange("cb p g yx -> p cb g yx"),
                                in_=zs[:],
                            )

            # ================= Phase B: layer-0 recurrence =================
            _recurrence(nc, tc, T if "B" in phases else 0, wh0_sb,
                        {d: bias_sb["0" + d] for d in ("f", "b")}, zx0, h0d, "l0")

            # ================= Phase C: layer-1 x projections =================
            h0pad = [
                pp.tile([128, GT, 5, 5], BF16, name=f"h0pad{par}", tag=f"h0pad{par}")
                for par in range(2)
            ]
            nc.gpsimd.memset(h0pad[0][:], 0.0)
            nc.gpsimd.memset(h0pad[1][:], 0.0)
            with (
                tc.tile_pool(name="c_h", bufs=3) as chp,
                tc.tile_pool(name="c_zp", bufs=5, space="PSUM") as zpp,
                tc.tile_pool(name="c_zs", bufs=3) as zsp,
            ):
                for gt in range(n_gt if "C" in phases else 0):
                    ga = gt * GT
                    hf_t = chp.tile([128, GT, 3, 3], F32, tag="hf")
                    hb_t = chp.tile([128, GT, 3, 3], F32, tag="hb")
                    nc.sync.dma_start(
                        out=hf_t[:].rearrange("p g y x -> p g (y x)"),
                        in_=h0d["f"][:, ga : ga + GT, :],
                    )
                    nc.sync.dma_start(
                        out=hb_t[:].rearrange("p g y x -> p g (y x)"),
                        in_=h0d["b"][:, ga : ga + GT, :],
                    )
                    hp = h0pad[gt % 2]
                    nc.vector.tensor_add(hp[:, :, 1:4, 1:4], hf_t[:], hb_t[:])
                    for sub in range(n_sub):
                        g0 = sub * 32
                        for d in ("f", "b"):
                            zs = zsp.tile([128, 4, 32, 9], F32, tag="zs")
                            for cb_o in range(4):
                                zp = zpp.tile([128, 32, 3, 3], F32, tag="zp")
                                zpf = zp[:].rearrange("p g y x -> p (g y x)")
                                for k, (dy, dx) in enumerate(TAPS):
                                    py, sy, ny = _clip(dy)
                                    px, sx, nx2 = _clip(dx)
                                    w_ap = wx1_sb[d][
                                        :, dy * 3 + dx,
                                        cb_o * 128 : (cb_o + 1) * 128,
                                    ]
                                    if CLIPPED and not (ny == 3 and nx2 == 3):
                                        o_ap = zp[:, :, py : py + ny, px : px + nx2]
                                        r_ap = hp[
                                            :, g0 : g0 + 32,
                                            1 + sy : 1 + sy + ny,
                                            1 + sx : 1 + sx + nx2,
                                        ]
                                    else:
                                        o_ap = zpf
                                        r_ap = hp[
                                            :, g0 : g0 + 32, dy : dy + 3, dx : dx + 3
                                        ]
                                    nc.tensor.matmul(
                                        o_ap, w_ap, r_ap,
                                        start=(k == 0),
                                        stop=(k == 8),
                                    )
                                nc.vector.tensor_copy(
                                    zs[:, cb_o],
                                    zp[:].rearrange("p g y x -> p g (y x)"),
                                )
                            nc.sync.dma_start(
                                out=zx1[d][
                                    :, :, ga + g0 : ga + g0 + 32, :
                                ].rearrange("cb p g yx -> p cb g yx"),
                                in_=zs[:],
                            )

            # ================= Phase D: layer-1 recurrence =================
            _recurrence(nc, tc, T if "D" in phases else 0, wh1_sb,
                        {d: bias_sb["1" + d] for d in ("f", "b")}, zx1, h1d, "l1")

            # ================= Phase E: FC head =================
            with (
                tc.tile_pool(name="e_h", bufs=3) as ehp,
                tc.tile_pool(name="e_ps", bufs=2, space="PSUM") as epp,
                tc.tile_pool(name="e_o", bufs=2) as eop,
            ):
                EC = min(128, G)  # groups per chunk
                assert G % EC == 0
                for gc in range(G // EC if "E" in phases else 0):
                    ga = gc * EC
                    hf_t = ehp.tile([128, EC, 9], F32, tag="ehf")
                    hb_t = ehp.tile([128, EC, 9], F32, tag="ehb")
                    h1s = ehp.tile([128, EC, 9], BF16, tag="eh1s")
                    nc.sync.dma_start(out=hf_t[:], in_=h1d["f"][:, ga : ga + EC, :])
                    nc.sync.dma_start(out=hb_t[:], in_=h1d["b"][:, ga : ga + EC, :])
                    nc.vector.tensor_add(h1s[:], hf_t[:], hb_t[:])
                    ps = epp.tile([NCLS, EC], F32, tag="eps")
                    for yx in range(9):
                        nc.tensor.matmul(
                            ps[:],
                            fcw_sb[:, yx, :],
                            h1s[:, :, yx],
                            start=(yx == 0),
                            stop=(yx == 8),
                        )
                    ot = eop.tile([NCLS, EC], F32, tag="eo")
                    nc.vector.tensor_scalar_add(ot[:], ps[:], fcb_sb[:, 0:1])
                    nc.sync.dma_start(out=out[:, ga : ga + EC], in_=ot[:])

    _orig_to_json = nc.to_json_bytes
    nc.to_json_bytes = lambda: _fix_multi_waits(_orig_to_json())
    return nc


def _recurrence(nc, tc, T, wh_sb, bias, zx, hout, name):
    """One bidirectional ConvLSTM recurrence. wh_sb/bias/zx/hout keyed by dir."""
    SIG = mybir.ActivationFunctionType.Sigmoid
    TANH = mybir.ActivationFunctionType.Tanh
    with (
        tc.tile_pool(name=f"{name}_st", bufs=1) as stp,
        tc.tile_pool(name=f"{name}_zx", bufs=6) as zxp,
        tc.tile_pool(name=f"{name}_ps", bufs=4, space="PSUM") as psp,
        tc.tile_pool(name=f"{name}_g", bufs=3) as gp,
    ):
        hpad = {}
        cst = {}
        for d in ("f", "b"):
            hpad[d] = [stp.tile([128, BL, 5, 5], BF16, name=f"{name}hp{d}{par}", tag=f"{name}hp{d}{par}") for par in range(2)]
            nc.gpsimd.memset(hpad[d][0][:], 0.0)
            nc.gpsimd.memset(hpad[d][1][:], 0.0)
            cst[d] = stp.tile([128, BL * 9], F32, name=f"{name}c{d}", tag=f"{name}c{d}")
            nc.gpsimd.memset(cst[d][:], 0.0)
        for s in range(T):
            for d in ("f", "b"):
                t = s if d == "f" else T - 1 - s
                hp_r = hpad[d][s % 2]
                hp_w = hpad[d][(s + 1) % 2]
                zxt = zxp.tile([128, 4, BL * 9], F32, name=f"zxt{d}", tag=f"zx{d}")
                nc.sync.dma_start(
                    out=zxt[:],
                    in_=zx[d][:, :, t * BL : (t + 1) * BL, :].rearrange(
                        "cb p b yx -> p cb (b yx)"
                    ),
                )
                zp = psp.tile([128, 4, BL * 9], F32, name=f"zp{d}", tag=f"zp{d}")
                for cb in range(4):
                    for k, (dy, dx) in enumerate(TAPS):
                        nc.tensor.matmul(
                            zp[:, cb],
                            wh_sb[d][:, dy * 3 + dx, cb * 128 : (cb + 1) * 128],
                            hp_r[:, :, dy : dy + 3, dx : dx + 3],
                            start=(k == 0),
                            stop=(k == 8),
                        )
                z = gp.tile([128, 4, BL * 9], F32, name=f"z{d}", tag=f"z{d}")
                nc.vector.tensor_add(z[:], zp[:], zxt[:])
                si = gp.tile([128, BL * 9], F32, name=f"si{d}", tag=f"si{d}")
                sf = gp.tile([128, BL * 9], F32, name=f"sf{d}", tag=f"sf{d}")
                so = gp.tile([128, BL * 9], F32, name=f"so{d}", tag=f"so{d}")
                tg = gp.tile([128, BL * 9], F32, name=f"tg{d}", tag=f"tg{d}")
                nc.scalar.activation(si[:], z[:, 0], SIG, bias=bias[d][:, 0:1])
                nc.scalar.activation(sf[:], z[:, 1], SIG, bias=bias[d][:, 1:2])
                nc.scalar.activation(so[:], z[:, 2], SIG, bias=bias[d][:, 2:3])
                nc.scalar.activation(tg[:], z[:, 3], TANH, bias=bias[d][:, 3:4])
                ig = gp.tile([128, BL * 9], F32, name=f"ig{d}", tag=f"ig{d}")
                nc.vector.tensor_mul(ig[:], si[:], tg[:])
                cf = gp.tile([128, BL * 9], F32, name=f"cf{d}", tag=f"cf{d}")
                nc.vector.tensor_mul(cf[:], sf[:], cst[d][:])
                nc.vector.tensor_add(cst[d][:], ig[:], cf[:])
                tcell = gp.tile([128, BL * 9], F32, name=f"tcl{d}", tag=f"tc{d}")
                nc.scalar.activation(tcell[:], cst[d][:], TANH)
                h = gp.tile([128, BL * 9], F32, name=f"h{d}", tag=f"h{d}")
                nc.vector.tensor_mul(h[:], so[:], tcell[:])
                nc.vector.tensor_copy(
                    hp_w[:, :, 1:4, 1:4],
                    h[:].rearrange("p (b y x) -> p b y x", y=3, x=3),
                )
                nc.sync.dma_start(
                    out=hout[d][:, t * BL : (t + 1) * BL, :].rearrange(
                        "p b yx -> p (b yx)"
                    ),
                    in_=h[:],
                )


# ---------------- host side ----------------

def _prep_weights(w, b, cin):
    """w: (512, cin+128, 3, 3) -> (wx, wh) bf16 host arrays + bias (128,4) f32."""
    bf = ml_dtypes.bfloat16
    wx = w[:, :cin].reshape(512, cin, 9)            # (co, ci, tap)
    wx = wx.transpose(1, 2, 0)                      # (ci, tap, co)
    if cin == 256:
        wx = wx.reshape(2, 128, 9, 512).transpose(1, 0, 2, 3)  # (128, 2, 9, 512)
    wx = np.ascontiguousarray(wx).astype(bf)
    wh = w[:, cin:].reshape(512, 128, 9).transpose(1, 2, 0)    # (128, 9, 512)
    wh = np.ascontiguousarray(wh).astype(bf)
    bias = np.ascontiguousarray(b.reshape(4, 128).T).astype(np.float32)
    return wx, wh, bias


def make_inputs_core(core, x, w_f0, b_f0, w_b0, b_b0, w_f1, b_f1, w_b1, b_b1,
                     fc_w, fc_b):
    m = {"x": np.ascontiguousarray(x[core * BL : (core + 1) * BL])}
    for d, w, b in (("f", w_f0, b_f0), ("b", w_b0, b_b0)):
        wx, wh, bias = _prep_weights(np.asarray(w), np.asarray(b), 256)
        m[f"wx0{d}"], m[f"wh0{d}"], m[f"bias0{d}"] = wx, wh, bias
    for d, w, b in (("f", w_f1, b_f1), ("b", w_b1, b_b1)):
        wx, wh, bias = _prep_weights(np.asarray(w), np.asarray(b), 128)
        m[f"wx1{d}"], m[f"wh1{d}"], m[f"bias1{d}"] = wx, wh, bias
    fcw = np.asarray(fc_w).reshape(NCLS, 128, 9).transpose(1, 2, 0)  # (128, 9, 7)
    m["fcw"] = np.ascontiguousarray(fcw).astype(ml_dtypes.bfloat16)
    m["fcb"] = np.ascontiguousarray(np.asarray(fc_b).reshape(NCLS, 1)).astype(np.float32)
    return m


_nc_cache = {}


def kernel(**inputs):
    from concourse.bass_utils import run_bass_kernel_spmd

    if "nc" not in _nc_cache:
        _nc_cache["nc"] = build_program(T_FULL)
    nc = _nc_cache["nc"]
    x = np.asarray(inputs["x"], dtype=np.float32)
    in_maps = [make_inputs_core(c, x, inputs["w_f0"], inputs["b_f0"],
                                inputs["w_b0"], inputs["b_b0"],
                                inputs["w_f1"], inputs["b_f1"],
                                inputs["w_b1"], inputs["b_b1"],
                                inputs["fc_w"], inputs["fc_b"])
               for c in range(NCORES)]
    res = run_bass_kernel_spmd(nc, in_maps, core_ids=list(range(NCORES)))
    outs = []
    for c in range(NCORES):
        o = res.results[c]["out"]  # (7, G) with g = t*BL + b
        o = o.reshape(NCLS, T_FULL, BL).transpose(2, 1, 0)  # (BL, T, 7)
        outs.append(o)
    return np.ascontiguousarray(np.concatenate(outs, axis=0), dtype=np.float32)

